# revision 29
# baseline (speedup 1.0000x reference)
"""Multi-head attention (b=2, n=2048, dim=1024, 16 heads x 64) on 8 TRN2 NeuronCores.

Sharding: core c handles batch c//4 and heads 4*(c%4) .. 4*(c%4)+3
(data parallel over batch x 4-way head/tensor parallel). w_qkv is
column-sharded by head; w_out is column-sharded: each core computes a
256-column slice of the output after AllGathers of the attention outputs
within its 4-core batch group (no all-reduce needed).

Device layout is feature-major ("K-major"): x arrives pre-transposed
[dim, n] in bf16; Q^T/K^T are produced feature-major and V token-major
directly from the QKV projection; attention scores are computed
transposed (dotsT[k, q]); softmax sums come from an augmented
ones-column in the V matmul; softmax exp runs on the scalar engine with
the 1/sqrt(d) scale folded in.

Schedule: one software-pipelined stream over 8 (head-pair, q-block)
blocks x 16 key-chunks. QKV projection and the output projection are
*woven into* the attention stream as work units with static emit steps
(the PE executes its stream in order, so emission order is the
schedule), which removes the serial QKV prologue and out-proj epilogue.
AllGathers fire per (head-pair, q-block) — 8 small collectives that
pipeline with attention; each out-proj unit consumes its gather ~12
steps later, and the last two out-proj units run in the drain so the PE
has work while the final AllGather flies. Softmax normalization uses
reciprocal_approx_fast on a packed [1,2,512] Z row and a gpsimd
partition_broadcast (no PE broadcast matmul, no full DVE reciprocal).
The final output is produced transposed [cols, n]; the host transposes
back.
"""

import sys

sys.path.insert(0, "/opt/trn_rl_repo")

import ml_dtypes
import numpy as np

import concourse.bass as bass  # noqa: F401  (engine types)
import concourse.tile as tile
from concourse import bacc, mybir
from concourse.bass_utils import run_bass_kernel_spmd

F32 = mybir.dt.float32
F32R = mybir.dt.float32r
BF16 = mybir.dt.bfloat16
NP_BF16 = np.dtype(ml_dtypes.bfloat16)

# Problem constants
B, N, DIM = 2, 2048, 1024
HEADS, DH = 16, 64
INNER = HEADS * DH
SCALE = DH ** -0.5
CORES = 8
GROUP_SIZE = 4
REPLICA_GROUPS = [[0, 1, 2, 3], [4, 5, 6, 7]]
HPC = 4  # heads per core
CS = HPC * DH  # 256 per-core feature columns

KC = DIM // 128  # 8 contraction chunks for dim
TT = N // 128  # 16 token tiles
QB = N // 512  # 4 q blocks
NKC = N // 128  # 16 key chunks
NBLK = 2 * QB  # 8 attention blocks (head-pair x q-block)


def build_nc():
    nc = bacc.Bacc("TRN2", target_bir_lowering=False, debug=False, num_devices=CORES)
    xt = nc.dram_tensor("xt", [DIM, N], BF16, kind="ExternalInput").ap()
    wq = nc.dram_tensor("wq", [DIM, CS], BF16, kind="ExternalInput").ap()
    wk = nc.dram_tensor("wk", [DIM, CS], BF16, kind="ExternalInput").ap()
    wv = nc.dram_tensor("wv", [DIM, CS], BF16, kind="ExternalInput").ap()
    wo = nc.dram_tensor("wo", [INNER, CS], BF16, kind="ExternalInput").ap()
    bo = nc.dram_tensor("bo", [CS], F32, kind="ExternalInput").ap()
    y = nc.dram_tensor("y", [CS, N], F32, kind="ExternalOutput").ap()  # y^T

    cc_in = [
        [nc.dram_tensor(f"cc_in{m}_{q}", [128, 512], BF16) for q in range(QB)]
        for m in range(2)
    ]
    cc_out = [
        [
            nc.dram_tensor(f"cc_out{m}_{q}", [GROUP_SIZE * 128, 512], BF16)
            for q in range(QB)
        ]
        for m in range(2)
    ]

    with tile.TileContext(nc) as tc:
        with (
            tc.tile_pool(name="big", bufs=2) as big,  # xt + the AG landing buffer
            tc.tile_pool(name="sb", bufs=1) as sb,
            tc.tile_pool(name="expp", bufs=4) as expp,
            tc.tile_pool(name="yout", bufs=3) as yout,
            tc.tile_pool(name="norm", bufs=4) as normp,
            tc.tile_pool(name="psd", bufs=2, space="PSUM") as psd,
            tc.tile_pool(name="pso", bufs=2, space="PSUM") as pso,
            tc.tile_pool(name="psy", bufs=2, space="PSUM") as psyp,
        ):
            # ---- load inputs (k/q weights + first token block first) -----
            xt_sb = big.tile([128, KC, N], BF16, tag="bigbuf")
            wq_sb = sb.tile([128, KC, CS], BF16)
            wk_sb = sb.tile([128, KC, CS], BF16)
            wv_sb = sb.tile([128, KC, CS], BF16)
            wo_sb = sb.tile([128, KC, CS], BF16)
            # wv + the first two token tiles land first so the V(0)/V(1)
            # prologue units give the PE work while the rest of qb0 streams
            # in for the K/Q projections.
            xt_r = xt.rearrange("(c p) n -> p c n", p=128)
            nc.sync.dma_start(out=wv_sb, in_=wv.rearrange("(c p) n -> p c n", p=128))
            nc.sync.dma_start(out=xt_sb[:, :, 0:128], in_=xt_r[:, :, 0:128])
            nc.sync.dma_start(out=xt_sb[:, :, 128:256], in_=xt_r[:, :, 128:256])
            nc.sync.dma_start(out=wk_sb, in_=wk.rearrange("(c p) n -> p c n", p=128))
            nc.sync.dma_start(out=xt_sb[:, :, 256:512], in_=xt_r[:, :, 256:512])
            nc.sync.dma_start(out=wq_sb, in_=wq.rearrange("(c p) n -> p c n", p=128))
            for qb in range(1, QB):
                sl = slice(qb * 512, (qb + 1) * 512)
                nc.sync.dma_start(out=xt_sb[:, :, sl], in_=xt_r[:, :, sl])
            nc.sync.dma_start(out=wo_sb, in_=wo.rearrange("(c p) n -> p c n", p=128))

            # bias, transposed layout: partition = column-within-block
            bias_sb = sb.tile([128, 2], F32)
            nc.sync.dma_start(out=bias_sb, in_=bo.rearrange("(cb p) -> p cb", p=128))

            ones_f = sb.tile([128, TT], F32)
            nc.vector.memset(ones_f, 1.0)

            # ---- persistent SBUF state ----------------------------------
            qt_sb = sb.tile([128, 2, N], BF16)
            kt_sb = sb.tile([128, 2, N], BF16)
            vaug = sb.tile([128, TT, HPC, DH + 1], BF16)
            with nc.allow_low_precision(reason="bf16 ones column"):
                for h in range(HPC):
                    nc.vector.tensor_copy(vaug[:, :, h, DH], ones_f)
            outt_sb = sb.tile([128, 2, N], BF16)
            y_acc = sb.tile([128, 2, N], F32)
            ag_all = big.tile([128, 2, QB, GROUP_SIZE, 512], BF16, tag="bigbuf")

            # ---- QKV / out-proj work units ------------------------------
            def unit_qk(hp, g, dst, w_sb):
                ps = psyp.tile([128, 512], F32, name="psy")
                for c in range(KC):
                    nc.tensor.matmul(
                        ps,
                        lhsT=w_sb[:, c, hp * 128 : (hp + 1) * 128],
                        rhs=xt_sb[:, c, g * 512 : (g + 1) * 512],
                        start=(c == 0),
                        stop=(c == KC - 1),
                    )
                with nc.allow_low_precision(reason="bf16 attention"):
                    nc.vector.tensor_copy(dst[:, hp, g * 512 : (g + 1) * 512], ps)

            def unit_v(t):
                ps = psyp.tile([128, 512], F32, name="psy")
                acc = ps[:, 0:CS]
                for c in range(KC):
                    nc.tensor.matmul(
                        acc,
                        lhsT=xt_sb[:, c, t * 128 : (t + 1) * 128],
                        rhs=wv_sb[:, c, :],
                        start=(c == 0),
                        stop=(c == KC - 1),
                    )
                with nc.allow_low_precision(reason="bf16 attention"):
                    nc.vector.tensor_copy(
                        vaug[:, t, :, 0:DH],
                        acc.rearrange("p (h d) -> p h d", d=DH),
                    )

            def unit_op(hp, qb):
                sl = slice(qb * 512, (qb + 1) * 512)
                for cb in range(2):
                    ps = psyp.tile([128, 512], F32, name="psy")
                    for c in range(4):
                        nc.tensor.matmul(
                            ps,
                            lhsT=wo_sb[:, hp * 4 + c, cb * 128 : (cb + 1) * 128],
                            rhs=ag_all[:, hp, qb, c, :],
                            start=(c == 0),
                            stop=(c == 3),
                        )
                    if hp == 0:
                        nc.vector.tensor_copy(y_acc[:, cb, sl], ps)
                    else:
                        y_sb = yout.tile([128, 512], F32, name="y_sb")
                        nc.vector.tensor_add(y_sb, ps, y_acc[:, cb, sl])
                        nc.vector.tensor_scalar_add(
                            out=y_sb, in0=y_sb, scalar1=bias_sb[:, cb : cb + 1]
                        )
                        nc.sync.dma_start(
                            out=y[cb * 128 : (cb + 1) * 128, sl], in_=y_sb
                        )

            # static weave: step -> units emitted after that step's dots/attV
            sched = {}

            def at(s, fn):
                sched.setdefault(s, []).append(fn)

            for t in range(2, TT):
                at(t, lambda t=t: unit_v(t))
            for g in range(1, QB):
                at(4 * g - 2, lambda g=g: unit_qk(0, g, kt_sb, wk_sb))
            for qb in range(1, QB):
                at(16 * qb - 4, lambda qb=qb: unit_qk(0, qb, qt_sb, wq_sb))
            # K(1,2)/K(1,3) are consumed from steps 72/76 — spread them into
            # block 4 so block 3 isn't unit-overloaded (PE-bound hump)
            for g, s_ in ((0, 50), (1, 54), (2, 66), (3, 70)):
                at(s_, lambda g=g: unit_qk(1, g, kt_sb, wk_sb))
            at(60, lambda: unit_qk(1, 0, qt_sb, wq_sb))
            for qb in range(1, QB):
                at(16 * qb + 59, lambda qb=qb: unit_qk(1, qb, qt_sb, wq_sb))
            # OP units run as LATE as their deadlines allow (~40 steps after
            # their AllGather fires): an early OP that catches a slow gather
            # head-of-line blocks the PE, and a >3.4us PE idle also drops the
            # HAM clock gate to 4/8 — a compounding penalty under rank skew.
            for qb, s_ in ((0, 56), (1, 80), (2, 88)):
                at(s_, lambda qb=qb: unit_op(0, qb))
            at(102, lambda: unit_op(1, 0))
            # OP(0,3), OP(1,1), OP(1,2), OP(1,3) run in the drain: their
            # data is long ready by then (except OP(1,3)) and they fill the
            # PE while the last AllGather flies.

            # ---- attention pipeline -------------------------------------
            def emit_dots(blk, kc):
                hp, qb = divmod(blk, QB)
                ps = psd.tile([128, 2, 512], F32, name="psd")
                for hh in range(2):
                    base = hh * DH
                    nc.tensor.matmul(
                        ps[:, hh, :],
                        lhsT=kt_sb[base : base + DH, hp, kc * 128 : (kc + 1) * 128],
                        rhs=qt_sb[base : base + DH, hp, qb * 512 : (qb + 1) * 512],
                        start=True,
                        stop=True,
                        tile_position=(base, 0),
                    )
                ex = expp.tile([128, 2, 512], BF16, name="expT")
                nc.scalar.activation(
                    out=ex, in_=ps, func=mybir.ActivationFunctionType.Exp, scale=SCALE
                )
                return ex

            def emit_attv(blk, kc, ex, po):
                hp = blk // QB
                for hh in range(2):
                    nc.tensor.matmul(
                        po[hh],
                        lhsT=vaug[:, kc, hp * 2 + hh, :],
                        rhs=ex[:, hh, :],
                        start=(kc == 0),
                        stop=(kc == NKC - 1),
                    )

            def finish_block(blk, po, last=False):
                hp, qb = divmod(blk, QB)
                sl = slice(qb * 512, (qb + 1) * 512)
                # evacuate po + Z row promptly so the pso slots free up
                # (for the last block nobody needs the slots — read PSUM
                # directly in the mul and skip the copies)
                # Z rows first: they gate the recip -> broadcast -> mul -> AG
                # chain; the po evacuation can overlap the reciprocal.
                zrow = normp.tile([1, 2, 512], F32, name="zrow")
                for hh in range(2):
                    nc.vector.tensor_copy(zrow[0:1, hh, :], po[hh][DH : DH + 1, :])
                zinv = normp.tile([1, 2, 512], F32, name="zinv")
                nc.vector.reciprocal_approx_fast(out=zinv[0:1, :, :], in_=zrow)
                po_sbs = []
                for hh in range(2):
                    if last:
                        po_sbs.append(po[hh][0:DH, :])
                    else:
                        po_sb = normp.tile([DH, 512], F32, name="po_sb")
                        nc.vector.tensor_copy(po_sb, po[hh][0:DH, :])
                        po_sbs.append(po_sb)
                for hh in range(2):
                    base = hh * DH
                    zb = normp.tile([DH, 512], F32, name="zb")
                    nc.gpsimd.partition_broadcast(zb, zinv[0:1, hh, :])
                    with nc.allow_low_precision(reason="bf16 attention out"):
                        nc.vector.tensor_mul(
                            outt_sb[base : base + DH, hp, sl], po_sbs[hh], zb
                        )
                nc.gpsimd.dma_start(out=cc_in[hp][qb].ap(), in_=outt_sb[:, hp, sl])
                nc.gpsimd.collective_compute(
                    "AllGather",
                    mybir.AluOpType.bypass,
                    ins=[cc_in[hp][qb].ap().opt()],
                    outs=[cc_out[hp][qb].ap().opt()],
                    replica_groups=REPLICA_GROUPS,
                )
                # land the gather per rank chunk: the out-proj matmul for
                # rank r only needs chunk r, so it can start as soon as the
                # first 128KB lands instead of after the full 512KB.
                cc_ap = cc_out[hp][qb].ap()
                for r in range(GROUP_SIZE):
                    nc.sync.dma_start(
                        out=ag_all[:, hp, qb, r, :],
                        in_=cc_ap[r * 128 : (r + 1) * 128, :],
                    )

            # prologue: V(0)/V(1) run while qb0 finishes streaming in, then
            # the K/Q groups that gate the first dots
            unit_v(0)
            unit_v(1)
            unit_qk(0, 0, kt_sb, wk_sb)
            unit_qk(0, 0, qt_sb, wq_sb)

            pend_attv = None
            po_cur = None
            po_prev = None
            for s in range(NBLK * NKC):
                blk, kc = divmod(s, NKC)
                if kc == 0:
                    po_prev = po_cur
                    po_cur = [
                        pso.tile([DH + 1, 512], F32, name="ps_o") for _ in range(2)
                    ]
                ex = emit_dots(blk, kc)
                if pend_attv is not None:
                    pblk, pkc, pex = pend_attv
                    emit_attv(pblk, pkc, pex, po_cur if pblk == blk else po_prev)
                    if pkc == NKC - 1:
                        finish_block(pblk, po_prev)
                pend_attv = (blk, kc, ex)
                for fn in sched.get(s, []):
                    fn()
            # drain
            pblk, pkc, pex = pend_attv
            emit_attv(pblk, pkc, pex, po_cur)
            finish_block(pblk, po_cur, last=True)
            unit_op(0, 3)
            unit_op(1, 1)
            unit_op(1, 2)
            unit_op(1, 3)

    nc.compile()
    return nc


_NC_CACHE = None


def _get_nc():
    global _NC_CACHE
    if _NC_CACHE is None:
        _NC_CACHE = build_nc()
    return _NC_CACHE


def _wo_perm(w_out):
    # chunk order [AG0: r0..r3 -> w_out rows 256r..256r+128,
    #              AG1: r0..r3 -> w_out rows 256r+128..256r+256]
    blocks = [w_out[256 * r : 256 * r + 128] for r in range(4)]
    blocks += [w_out[256 * r + 128 : 256 * r + 256] for r in range(4)]
    return np.concatenate(blocks, axis=0)


def _make_in_maps(x, w_qkv, w_out, b_out):
    wop = _wo_perm(w_out)
    in_maps = []
    for c in range(CORES):
        bi = c // GROUP_SIZE
        g = c % GROUP_SIZE
        cols = slice(g * CS, (g + 1) * CS)
        in_maps.append(
            {
                "xt": np.ascontiguousarray(x[bi].T).astype(NP_BF16),
                "wq": np.ascontiguousarray(w_qkv[:, cols]).astype(NP_BF16),
                "wk": np.ascontiguousarray(w_qkv[:, INNER:][:, cols]).astype(NP_BF16),
                "wv": np.ascontiguousarray(w_qkv[:, 2 * INNER:][:, cols]).astype(
                    NP_BF16
                ),
                "wo": np.ascontiguousarray(wop[:, cols]).astype(NP_BF16),
                "bo": np.ascontiguousarray(b_out[cols]),
            }
        )
    return in_maps


def _assemble(results):
    out = np.empty((B, N, DIM), dtype=np.float32)
    for c in range(CORES):
        bi = c // GROUP_SIZE
        g = c % GROUP_SIZE
        out[bi, :, g * CS : (g + 1) * CS] = results[c]["y"].T
    return out


def kernel(x, w_qkv, w_out, b_out, _trace=False, _trace_kwargs=None):
    x = np.asarray(x, dtype=np.float32)
    w_qkv = np.asarray(w_qkv, dtype=np.float32)
    w_out = np.asarray(w_out, dtype=np.float32)
    b_out = np.asarray(b_out, dtype=np.float32)
    nc = _get_nc()
    in_maps = _make_in_maps(x, w_qkv, w_out, b_out)
    res = run_bass_kernel_spmd(
        nc,
        in_maps,
        core_ids=list(range(CORES)),
        trace=_trace,
        **(_trace_kwargs or {}),
    )
    out = _assemble(res.results)
    if _trace:
        return out, res
    return out
